# revision 5
# baseline (speedup 1.0000x reference)
"""GCN layer v3: baseline one-hot-scatter structure, fp16 data path.

Changes vs baseline kernel.py:
  - features are cast f32 -> fp16 on device into an Internal DRAM table
    (196 blocks of 512 rows: DMA in, DVE cast, DMA out), halving gather
    DMA traffic.
  - dma_gather fetches 256B fp16 rows; G tiles fp16.
  - one-hot S tiles and iota are fp16 (DVE is_eq ~1.5x faster, PE matmul
    single-pass instead of fp32 double-pass).
  - PSUM h^T accumulation, htile adds, and the final h @ W + b / ReLU
    stage stay f32 exactly as the baseline.
"""

import os

import numpy as np

import concourse.bacc as bacc
import concourse.mybir as mybir
import concourse.tile as tile
from concourse import bass_utils
from concourse.bass import _bass_rust


def _ensure_ntff_hook():
    """Install the axon NTFF profile hook shim if the env lacks one.

    concourse's trace path imports antenv.axon_hooks, which some agent
    images don't ship; without it trace=True raises. Harmless no-op when
    the real module (or a prior shim) exists.
    """
    try:
        import antenv.axon_hooks  # noqa: F401
        return
    except ImportError:
        pass
    try:
        import sys
        import types

        import antenv
        from trn_agent_boot.trn_boot import _ntff_profile_via_ctypes

        mod = types.ModuleType("antenv.axon_hooks")
        mod._hook = _ntff_profile_via_ctypes("/opt/axon/libaxon_pjrt.so")
        mod.set_axon_ntff_profile_hook = lambda h: setattr(mod, "_hook", h)
        mod.get_axon_ntff_profile_hook = lambda: mod._hook
        sys.modules["antenv.axon_hooks"] = mod
        antenv.axon_hooks = mod
    except Exception:
        pass

P = 128
D = 128
F = 128
N_NODES = 100000
N_CORES = 8
NPC = N_NODES // N_CORES            # 12500
NPC_PAD = ((NPC + P - 1) // P) * P  # 12544
N_TILES = NPC_PAD // P              # 98
N_WIN = 4
WIN_ROWS = N_NODES // N_WIN         # 25000 (< int16 max)
SUP = int(os.environ.get("GCN_SUP", "384"))  # dst super-tile width
N_SUP = (NPC_PAD + SUP - 1) // SUP  # 25 (last one 256 wide)
PRE_BLK = int(os.environ.get("GCN_PREBLK", "2048"))  # preamble rows per block

CALL_COLS = int(os.environ.get("GCN_CALLCOLS", "12"))
N_QUEUES = int(os.environ.get("GCN_NQ", "3"))
NQ_ALLOC = int(os.environ.get("GCN_NQALLOC", "4"))
SCRATCH = int(os.environ.get("GCN_SCRATCH", "16384"))


def _sup_width(ts):
    return min(SUP, NPC_PAD - ts * SUP)


def _build_schedule(edge_src, edge_dst):
    """Shared column schedule + per-core index/dst streams (as baseline)."""
    core_of = edge_dst // NPC
    counts = np.zeros((N_CORES, N_WIN, N_SUP), np.int64)
    per_core_raw = []
    for k in range(N_CORES):
        m = core_of == k
        dstl = (edge_dst[m] - k * NPC).astype(np.int64)
        src = edge_src[m].astype(np.int64)
        w = src // WIN_ROWS
        t = dstl // SUP
        np.add.at(counts[k], (w, t), 1)
        per_core_raw.append((dstl, src, w, t))

    ncols = (counts.max(axis=0) + P - 1) // P      # [N_WIN, N_SUP]
    tile_tot = ncols.sum(axis=0)
    ncols[0] = np.where(tile_tot == 0, 1, ncols[0])

    flat = ncols.reshape(-1)
    off_flat = np.concatenate([[0], np.cumsum(flat)])
    col_off = off_flat[:-1].reshape(N_WIN, N_SUP)
    total_cols = int(off_flat[-1])

    calls = []  # (window, col_start, col_end) — packed to CALL_COLS
    for w in range(N_WIN):
        cur = int(col_off[w, 0])
        end = int(col_off[w, N_SUP - 1] + ncols[w, N_SUP - 1])
        while cur < end:
            nxt = min(cur + CALL_COLS, end)
            calls.append((w, cur, nxt))
            cur = nxt

    per_core = []
    for k in range(N_CORES):
        dstl, src, w, t = per_core_raw[k]
        key = w * N_SUP + t
        order = np.argsort(key, kind="stable")
        key_s = key[order]
        grp_start = np.concatenate([[0], np.cumsum(np.bincount(
            key_s, minlength=N_WIN * N_SUP))])[:-1]
        pos_in_grp = np.arange(key_s.size) - grp_start[key_s]
        flatpos = off_flat[key_s] * P + pos_in_grp

        gidx = np.zeros(total_cols * P, np.int16)
        drel = np.full(total_cols * P, -1.0, np.float32)
        gidx[flatpos] = (src[order] - w[order] * WIN_ROWS).astype(np.int16)
        drel[flatpos] = (dstl[order] - t[order] * SUP).astype(np.float32)

        idx_pm = np.zeros((P, total_cols * 8), np.int16)
        for (_w, c0, c1) in calls:
            seg = gidx[c0 * P:c1 * P]
            idx_pm[:, c0 * 8:c1 * 8] = np.tile(seg.reshape(-1, 16).T, (8, 1))
        drel_pm = np.ascontiguousarray(drel.reshape(total_cols, P).T)
        per_core.append((idx_pm, drel_pm))

    return ncols, col_off, total_cols, calls, per_core


def _build_module(ncols, col_off, total_cols, calls, repeat=1):
    f32 = mybir.dt.float32
    f16 = mybir.dt.float16
    i16 = mybir.dt.int16
    nc = bacc.Bacc(
        "TRN2", target_bir_lowering=False, debug=False,
        num_devices=N_CORES, num_swdge_queues=max(NQ_ALLOC, N_QUEUES, 1),
        dynamic_dma_scratch_size=SCRATCH,
    )
    feats = nc.dram_tensor("features", [N_NODES, D], f32, kind="ExternalInput")
    feats16 = nc.dram_tensor("feats16", [N_NODES, D], f16, kind="Internal")
    ell = nc.dram_tensor("ell_idx", [P, total_cols * 8], i16,
                         kind="ExternalInput")
    drel_d = nc.dram_tensor("dstrel", [P, total_cols], f32,
                            kind="ExternalInput")
    iota_d = nc.dram_tensor("iota", [P, SUP], f16, kind="ExternalInput")
    ones_d = nc.dram_tensor("ones", [1, P], f32, kind="ExternalInput")
    w_d = nc.dram_tensor("W", [D, F], f32, kind="ExternalInput")
    b_d = nc.dram_tensor("b", [1, F], f32, kind="ExternalInput")
    out_d = nc.dram_tensor("out", [NPC_PAD, F], f32, kind="ExternalOutput")
    out_v = out_d[:].rearrange("(t p) f -> t p f", p=P)
    pre_r = PRE_BLK // 128
    feats_v = feats[:].rearrange("(q r) d -> q (r d)", r=pre_r)
    feats16_v = feats16[:].rearrange("(q r) d -> q (r d)", r=pre_r)
    n_blk = (N_NODES + PRE_BLK - 1) // PRE_BLK

    def call_groups(w, c0, c1):
        groups = []
        for t in range(N_SUP):
            s = max(int(col_off[w, t]), c0)
            e = min(int(col_off[w, t] + ncols[w, t]), c1)
            if e > s:
                groups.append((t, list(range(s, e))))
        return groups

    with tile.TileContext(nc) as tc:
        with (
            tc.tile_pool(name="const", bufs=1) as cpool,
            tc.tile_pool(name="ht", bufs=1) as htpool,
            tc.tile_pool(name="pre", bufs=6) as prepool,
            tc.tile_pool(name="G", bufs=3) as gpool,
            tc.tile_pool(name="S", bufs=10) as spool,
            tc.tile_pool(name="stage", bufs=2) as stpool,
            tc.tile_pool(name="hps", bufs=4, space="PSUM") as hps,
            tc.tile_pool(name="ops", bufs=2, space="PSUM") as ops,
        ):
            idx_sb = cpool.tile([P, total_cols * 8], i16)
            nc.sync.dma_start(out=idx_sb[:], in_=ell[:])
            drel_sb = cpool.tile([P, total_cols], f32)
            nc.sync.dma_start(out=drel_sb[:], in_=drel_d[:])
            iota_sb = cpool.tile([P, SUP], f16)
            nc.sync.dma_start(out=iota_sb[:], in_=iota_d[:])
            ones_sb = cpool.tile([1, P], f32)
            nc.sync.dma_start(out=ones_sb[:], in_=ones_d[:])
            w_sb = cpool.tile([D, F], f32)
            nc.sync.dma_start(out=w_sb[:], in_=w_d[:])
            b_sb = cpool.tile([1, F], f32)
            nc.sync.dma_start(out=b_sb[:], in_=b_d[:])

            # preamble: cast features f32 -> fp16 in DRAM
            win_pre_dmas = [[] for _ in range(N_WIN)]
            for blk in range(n_blk):
                a = blk * PRE_BLK
                nb = min(PRE_BLK, N_NODES - a)
                q = nb // pre_r
                fin = prepool.tile([P, PRE_BLK], f32, tag="fin",
                                   name=f"fin{blk}")
                f16t = prepool.tile([P, PRE_BLK], f16, tag="f16",
                                    name=f"f16_{blk}")
                nc.sync.dma_start(out=fin[:q, :], in_=feats_v[a // pre_r:a // pre_r + q, :])
                nc.scalar.activation(
                    out=f16t[:q, :], in_=fin[:q, :],
                    func=mybir.ActivationFunctionType.Copy)
                dout = nc.sync.dma_start(
                    out=feats16_v[a // pre_r:a // pre_r + q, :],
                    in_=f16t[:q, :])
                w_lo = a // WIN_ROWS
                w_hi = min((a + nb - 1) // WIN_ROWS, N_WIN - 1)
                for w in range(w_lo, w_hi + 1):
                    win_pre_dmas[w].append(dout)

            for rep in range(repeat):
                htile = {}
                n_seen = {}
                win_dep_done = set()
                for ci, (w, c0, c1) in enumerate(calls):
                    cc = c1 - c0
                    g = gpool.tile([P, cc * D], f16, tag=f"G{ci % 2}",
                                   name=f"g_{rep}_{ci}")
                    gi = nc.gpsimd.dma_gather(
                        out_ap=g[:].rearrange("p (c d) -> p c d", d=D),
                        in_ap=feats16[w * WIN_ROWS:(w + 1) * WIN_ROWS, :],
                        idxs_ap=idx_sb[:, c0 * 8:c1 * 8],
                        num_idxs=cc * P,
                        num_idxs_reg=cc * P,
                        elem_size=D,
                        single_packet=False,
                        queue_num=ci % max(N_QUEUES, 1),
                    )
                    if w not in win_dep_done:
                        win_dep_done.add(w)
                        for dout in win_pre_dmas[w]:
                            _bass_rust.add_dep_helper(
                                gi.ins, dout.ins,
                                reason="gather waits fp16 cast preamble")
                    for t, cols in call_groups(w, c0, c1):
                        sw = _sup_width(t)
                        acc = hps.tile([P, SUP], mybir.dt.float32, tag="hps",
                                       name=f"acc_{rep}_{w}_{t}")
                        for j, c in enumerate(cols):
                            s = spool.tile([P, SUP], f16, tag="S",
                                           name=f"s_{rep}_{c}")
                            nc.vector.tensor_scalar(
                                out=s[:, :sw], in0=iota_sb[:, :sw],
                                scalar1=drel_sb[:, c:c + 1], scalar2=None,
                                op0=mybir.AluOpType.is_equal,
                            )
                            nc.tensor.matmul(
                                out=acc[:, :sw],
                                lhsT=g[:, (c - c0) * D:(c - c0 + 1) * D],
                                rhs=s[:, :sw],
                                start=(j == 0),
                                stop=(j == len(cols) - 1),
                            )
                        if t not in htile:
                            htile[t] = htpool.tile(
                                [P, SUP], f32, tag=f"ht{t}", name=f"ht{t}")
                            nc.scalar.activation(
                                out=htile[t][:, :sw], in_=acc[:, :sw],
                                func=mybir.ActivationFunctionType.Copy,
                            )
                        else:
                            nc.vector.tensor_tensor(
                                out=htile[t][:, :sw], in0=htile[t][:, :sw],
                                in1=acc[:, :sw], op=mybir.AluOpType.add,
                            )
                        n_seen[t] = n_seen.get(t, 0) + len(cols)
                        if n_seen[t] == int(ncols[:, t].sum()):
                            for tt in range(t * SUP // P,
                                            min((t * SUP + sw) // P, N_TILES)):
                                o = (tt * P) % SUP
                                o_ps = ops.tile([P, F], mybir.dt.float32,
                                                tag="ops",
                                                name=f"ops_{rep}_{tt}")
                                nc.tensor.matmul(
                                    out=o_ps[:], lhsT=htile[t][:, o:o + P],
                                    rhs=w_sb[:], start=True, stop=False)
                                nc.tensor.matmul(
                                    out=o_ps[:], lhsT=ones_sb[:], rhs=b_sb[:],
                                    start=False, stop=True)
                                stage = stpool.tile([P, F], f32, tag="stage",
                                                    name=f"st_{rep}_{tt}")
                                nc.scalar.activation(
                                    out=stage[:], in_=o_ps[:],
                                    func=mybir.ActivationFunctionType.Relu,
                                )
                                nc.sync.dma_start(out=out_v[tt], in_=stage[:])
    nc.compile()
    return nc


_CACHE: dict = {}


def _get_module(edge_src, edge_dst, repeat=1):
    key = (hash((edge_src.tobytes(), edge_dst.tobytes())), repeat)
    if _CACHE.get("key_" + str(repeat)) == key:
        return _CACHE["val_" + str(repeat)]
    if _CACHE.get("sched_key") == key[0]:
        sched = _CACHE["sched"]
    else:
        sched = _build_schedule(edge_src, edge_dst)
        _CACHE["sched_key"] = key[0]
        _CACHE["sched"] = sched
    ncols, col_off, total_cols, calls, per_core = sched
    nc = _build_module(ncols, col_off, total_cols, calls, repeat=repeat)
    _CACHE["key_" + str(repeat)] = key
    _CACHE["val_" + str(repeat)] = (nc, per_core)
    return _CACHE["val_" + str(repeat)]


def _in_maps(features, W, b, per_core):
    iota = np.ascontiguousarray(
        np.broadcast_to(np.arange(SUP, dtype=np.float16), (P, SUP)))
    ones = np.ones((1, P), np.float32)
    maps = []
    for k in range(N_CORES):
        idx_pm, drel_pm = per_core[k]
        maps.append({
            "features": features,
            "ell_idx": idx_pm,
            "dstrel": drel_pm,
            "iota": iota,
            "ones": ones,
            "W": W,
            "b": b,
        })
    return maps


def kernel(features, W, b, edge_src, edge_dst):
    features = np.ascontiguousarray(np.asarray(features), dtype=np.float32)
    W = np.ascontiguousarray(np.asarray(W), dtype=np.float32)
    b = np.ascontiguousarray(np.asarray(b), dtype=np.float32).reshape(1, F)
    edge_src = np.asarray(edge_src).astype(np.int64)
    edge_dst = np.asarray(edge_dst).astype(np.int64)

    repeat = int(os.environ.get("GCN_REPEAT", "1"))
    nc, per_core = _get_module(edge_src, edge_dst, repeat=repeat)

    do_trace = bool(int(os.environ.get("GCN_TRACE", "0")))
    if do_trace:
        _ensure_ntff_hook()
    res = bass_utils.run_bass_kernel_spmd(
        nc, _in_maps(features, W, b, per_core),
        core_ids=list(range(N_CORES)),
        trace=do_trace,
    )
    if res.exec_time_ns is not None:
        print(f"HW exec time: {res.exec_time_ns} ns")
    _CACHE["last_res"] = res

    out = np.empty((N_NODES, F), np.float32)
    for k in range(N_CORES):
        out[k * NPC:(k + 1) * NPC] = res.results[k]["out"][:NPC]
    return out
